# revision 33
# baseline (speedup 1.0000x reference)
"""Trainium2 Bass kernel for nn_GroupedLossWithIndexMap.

Reference computation (per batch item b, N=65536 rows, C_old=128, C_new=16):
    probs   = softmax(inputs[b], axis=-1)            # [N, 128]
    grouped = probs @ GROUP_MAT                      # [N, 16] (8 contiguous cols per group)
    avg     = mean(grouped, axis=0)                  # [16]
    loss_b  = KL(softmax(targets[b]/100) || softmax(avg)) / 16
    out     = mean_b(loss_b)

Key identity: grouping+mean commute, so each core only needs
    colsum[c] = sum_n exp(x[n,c]) / rowsum[n]        # [128]
and the rest is trivial scalar math done on host.

Device kernel (per core, one batch item, data parallel over 8 cores):
  - DMA: 7 bulk groups of 4 MiB (16 KB/partition lines, ~415 GB/s) + a
    tapered tail of small groups so the compute pipeline drains with the
    data. Taper groups use dedicated SBUF buffers so their DMA triggers
    carry no buffer-recycle waits and all issue up-front.
  - ACT: exp (f32 -> bf16) in 8-row units.
  - DVE: row sums via single direct [128,8,128] reduces (634ns each —
    cheaper than any add-tree at this granularity), one reciprocal per
    group.
  - PE : per 4-row chunk, psum[0:4, 0:512] += rb^T @ exp (diagonal blocks
    hold the real reciprocal-weighted colsums). Two accumulation chains:
    bank A (bulk) closes early so its PSUM copy + output DMA overlap the
    taper; bank B (taper) ships at the end.
  - out: colsum [4, 2*4*128] f32 -> DRAM; host sums the two banks'
    diagonal blocks and finishes the tiny KL math in numpy.
"""

import numpy as np

B = 8
N = 65536
C = 128
G = 16
P = 128
K = 32          # rows per partition per group tile
NG = N // (P * K)   # 16 groups
EPS = 1e-8

_compiled = None


def _patch_tile_epilogue(tile):
    """Replace TileContext's end-of-kernel drain+barrier with a lighter one:
    the sync.drain already waits on the global completion clock, so the two
    all-engine barriers around the semaphore clears only need sequencer-level
    (sem_only) sync — the per-engine InstDrains they normally emit cost ~9us.
    The semaphore clear itself is kept: it is what guarantees a clean device
    for the next execution (removing it was measured to occasionally corrupt
    the FIRST run of a fresh process after a dirty predecessor — NaN)."""
    if getattr(tile.TileContext, "_fast_epilogue", False):
        return
    from concourse.vector_clock import ScopedClock

    def _drain_and_barrier(self, tick_clock, wait_clock):
        drain_inst = self.nc.sync.drain()
        wait_clock.add_sem_waits(
            drain_inst.ins, ScopedClock({None: tick_clock.global_clock})
        )
        self.nc.all_engine_barrier(sem_only=True)
        popped = self.nc._tile_sem_poison_stack.pop()
        assert popped is self._sem_poison
        self.nc.clear_and_free_semaphores(list(self.sems.allocated().values()))
        self.nc.all_engine_barrier(sem_only=True)

    tile.TileContext._drain_and_barrier = _drain_and_barrier
    tile.TileContext._fast_epilogue = True


def _build(ng: int = NG):
    import concourse.bacc as bacc
    import concourse.bass as bass
    import concourse.tile as tile
    from concourse import mybir

    _patch_tile_epilogue(tile)

    f32 = mybir.dt.float32
    bf16 = mybir.dt.bfloat16

    n = P * K * ng

    nc = bacc.Bacc(
        "TRN2",
        target_bir_lowering=False,
        debug=False,
        num_devices=B,
    )

    x = nc.dram_tensor("x", [n, C], f32, kind="ExternalInput")
    colsum = nc.dram_tensor("colsum", [4, 2 * 4 * C], f32, kind="ExternalOutput")

    # DMA group schedule: bulk groups of K rows/partition, then a gentle
    # taper so the compute pipeline (exp -> rowsum -> matmul) drains along
    # with the data. Compute is further split into units of <= KU rows so
    # each pipeline stage's per-unit latency is small: the serial drain
    # after the last byte lands is one small unit's chain, not a 2 MiB
    # group's worth of exp+reduce+matmul.
    # 2 MiB bulk groups: 16 KB/partition lines already run at the ~415 GB/s
    # roofline, and the smaller group keeps exp's start latency (it waits
    # for the whole group's DMA) at ~5us instead of ~10us, which shrinks
    # the compute backlog at the bulk/taper boundary.
    KB = 32
    # Taper: eight uniform 8-row groups. Same total DMA time as a graded
    # taper (0.5 MiB transfers run ~270-400 GB/s), but every taper group is
    # a single 8-row compute unit with its own reciprocal — fewest serial
    # ACT/DVE steps after the bulk boundary.
    if ng == NG:
        specs = [KB] * 14 + [8] * 8
    else:
        specs = [K] * ng
    assert sum(specs) == ng * K

    MB = 4   # matmul chunk block: MB chunks of 128 rows per matmul instruction
    # 8-row compute units: a direct [128,8,128] reduce costs 634ns, so two
    # of them (1.27us/16 rows) beat the 16-row add-tree (~1.74us) — the
    # add-tree path below becomes dead and total DVE work drops ~25%.
    KU = 8

    # compute units: (group, offset_rows, unit_rows)
    units = []
    for g, kk in enumerate(specs):
        o = 0
        while o < kk:
            u = min(KU, kk - o)
            units.append((g, o, u))
            o += u
    # Taper-group units accumulate into PSUM bank B so bank A's chain closes
    # at the end of the bulk phase: its PSUM->SBUF copy and output DMA then
    # overlap the taper instead of queueing at the very end.
    nbulk = sum(1 for g, kk in enumerate(specs) if kk == KB)
    tail_unit = lambda g: g >= nbulk
    nA = sum(max(1, u // MB) for (g, _, u) in units if not tail_unit(g))
    nB = sum(max(1, u // MB) for (g, _, u) in units if tail_unit(g))

    with tile.TileContext(nc) as tc:
        with (
            tc.tile_pool(name="xin", bufs=6) as xpool,
            tc.tile_pool(name="xtap", bufs=1) as tpool,
            tc.tile_pool(name="exp", bufs=12) as epool,
            tc.tile_pool(name="half", bufs=4) as hpool,
            tc.tile_pool(name="small", bufs=12) as spool,
            tc.tile_pool(name="out", bufs=1) as opool,
            tc.tile_pool(name="psum", bufs=2, space="PSUM") as ppool,
        ):
            psA = ppool.tile([MB, MB * C], f32, tag="psA")
            psB = ppool.tile([MB, MB * C], f32, tag="psB")
            ot = opool.tile([MB, 2 * MB * C], f32)

            xts = {}
            row0 = 0
            for g, kk in enumerate(specs):
                src = (
                    x.ap()[row0 : row0 + P * kk, :]
                    .rearrange("(p k) c -> p (k c)", p=P, k=kk)
                )
                if kk == KB:
                    # Two 1 MiB half-DMAs per group: the overlap tracker is
                    # slice-precise, so each 16-row exp unit depends only on
                    # its own half — exp starts ~2.5us earlier than with one
                    # whole-group transfer, shrinking the ACT backlog that
                    # gates the post-stream tail.
                    xt = xpool.tile([P, kk * C], f32, tag="x")
                    h = (kk // 2) * C
                    nc.sync.dma_start(out=xt[:, 0:h], in_=src[:, 0:h])
                    nc.sync.dma_start(out=xt[:, h : kk * C], in_=src[:, h : kk * C])
                else:
                    # Taper groups get dedicated buffers so their DMA
                    # triggers carry no buffer-recycle waits: all issue
                    # up-front and the SDMA queue streams bulk -> taper
                    # back-to-back with no issue-side gaps.
                    xt = tpool.tile([P, kk * C], f32, tag=f"t{g}")
                    nc.sync.dma_start(out=xt[:], in_=src)
                xts[g] = xt
                row0 += P * kk

            mmiA = mmiB = 0
            gr_st = gr_rb = None
            gr_first = False
            for ui, (g, o, u) in enumerate(units):
                xt = xts[g]
                kk = specs[g]
                tail = tail_unit(g)
                et = epool.tile([P, u * C], bf16, tag="e")
                nc.scalar.activation(
                    et[:],
                    xt[:, o * C : (o + u) * C],
                    mybir.ActivationFunctionType.Exp,
                )

                e3 = et[:].rearrange("p (k c) -> p k c", c=C)
                # One st/rb tile per GROUP: each unit's reduce writes its
                # slice, then a single reciprocal covers the whole group
                # (reciprocal has ~200ns fixed cost; batching halves it).
                if o == 0:
                    gr_st = spool.tile([P, kk], f32, tag="s")
                    gr_rb = spool.tile([P, kk], bf16, tag="rb")
                st = gr_st[:, o : o + u]
                if u <= 8:
                    # small unit: single reduce beats 3 instructions
                    nc.vector.reduce_sum(st, e3, axis=mybir.AxisListType.X)
                else:
                    at = hpool.tile([P, u * 64], bf16, tag="a")
                    a3 = at[:].rearrange("p (k c) -> p k c", c=64)
                    nc.vector.tensor_add(a3, e3[:, :, 0:64], e3[:, :, 64:128])
                    bt = hpool.tile([P, u * 32], bf16, tag="b")
                    b3 = bt[:].rearrange("p (k c) -> p k c", c=32)
                    nc.vector.tensor_add(b3, a3[:, :, 0:32], a3[:, :, 32:64])
                    nc.vector.reduce_sum(st, b3, axis=mybir.AxisListType.X)
                if o + u == kk:
                    with nc.allow_low_precision("bf16 reciprocal weights"):
                        nc.vector.reciprocal(gr_rb[:], gr_st[:])

                for k0 in range(o, o + u, MB):
                    m = min(MB, o + u - k0)
                    if tail:
                        nc.tensor.matmul(
                            psB[0:m, 0 : m * C],
                            gr_rb[:, k0 : k0 + m],
                            et[:, (k0 - o) * C : (k0 - o + m) * C],
                            start=(mmiB == 0),
                            stop=(mmiB == nB - 1),
                        )
                        mmiB += 1
                    else:
                        nc.tensor.matmul(
                            psA[0:m, 0 : m * C],
                            gr_rb[:, k0 : k0 + m],
                            et[:, (k0 - o) * C : (k0 - o + m) * C],
                            start=(mmiA == 0),
                            stop=(mmiA == nA - 1),
                        )
                        mmiA += 1
                if mmiA == nA and not tail:
                    # Bank A's chain closed at the end of the bulk phase:
                    # copy+ship it now, overlapped with the taper. ACT does
                    # the PSUM read — DVE is the busy engine here.
                    nc.scalar.copy(ot[:, 0 : MB * C], psA[:])
                    nc.sync.dma_start(
                        out=colsum[:, 0 : MB * C], in_=ot[:, 0 : MB * C]
                    )
            assert mmiA == nA and mmiB == nB, (mmiA, nA, mmiB, nB)

            # Diagonal blocks of psA/psB hold the real partial colsums;
            # off-diagonal blocks are accumulation garbage. Ship both banks
            # and let the host pick the diagonals (engines can't start at
            # partition>0, and DMA can't read PSUM directly).
            nc.scalar.copy(ot[:, MB * C : 2 * MB * C], psB[:])
            nc.sync.dma_start(
                out=colsum[:, MB * C : 2 * MB * C], in_=ot[:, MB * C : 2 * MB * C]
            )

    nc.compile()
    return nc


def _get_compiled():
    global _compiled
    if _compiled is None:
        _compiled = _build()
    return _compiled


def _run_device(inputs: np.ndarray, trace: bool = False, **kwargs):
    from concourse.bass_utils import run_bass_kernel_spmd

    nc = _get_compiled()
    in_maps = [
        {"x": np.ascontiguousarray(inputs[i], dtype=np.float32)} for i in range(B)
    ]
    res = run_bass_kernel_spmd(nc, in_maps, list(range(B)), trace=trace, **kwargs)
    # colsum is [4, 2*4*C]: two PSUM banks side by side, each [4, 4*C] whose
    # 4 diagonal [1, C] blocks are real partial colsums.
    colsums = np.stack(
        [
            np.asarray(res.results[i]["colsum"], dtype=np.float64)
            .reshape(4, 2, 4, C)[np.arange(4), :, np.arange(4)]
            .sum(axis=(0, 1))
            for i in range(B)
        ]
    )  # [B, 128]
    return colsums, res


def _finish_host(colsums: np.ndarray, targets: np.ndarray) -> np.ndarray:
    # colsums: [B, 128] float; targets: [B, 16]
    cs = colsums.astype(np.float64)
    avg = cs.reshape(B, G, C // G).sum(axis=-1) / N          # [B, 16]
    # softmax(avg)
    a = avg - avg.max(axis=-1, keepdims=True)
    p = np.exp(a)
    p /= p.sum(axis=-1, keepdims=True)
    # softmax(targets / 100)
    t = targets.astype(np.float64) / 100.0
    t = t - t.max(axis=-1, keepdims=True)
    t = np.exp(t)
    t /= t.sum(axis=-1, keepdims=True)
    log_p = np.log(p + EPS)
    kl = (t * (np.log(t) - log_p)).sum(axis=-1) / G          # [B]
    return np.float32(kl.mean())


def kernel(inputs: np.ndarray, targets: np.ndarray) -> np.ndarray:
    inputs = np.asarray(inputs)
    colsums = None
    for _attempt in range(3):
        colsums, _ = _run_device(inputs)
        # Invariant: sum_c colsum[c] = sum_n sum_c exp(x)/rowsum = N exactly
        # (up to bf16 rounding, ~0.5%). The first execution after a dirty
        # device state (killed predecessor / congested upload) occasionally
        # returns garbage or NaN — detect and retry.
        tot = colsums.sum(axis=1)
        if np.all(np.isfinite(tot)) and np.all(np.abs(tot - N) < 0.05 * N):
            break
    return _finish_host(colsums, np.asarray(targets))



# revision 34
# speedup vs baseline: 1.0066x; 1.0066x over previous
"""Trainium2 Bass kernel for nn_GroupedLossWithIndexMap.

Reference computation (per batch item b, N=65536 rows, C_old=128, C_new=16):
    probs   = softmax(inputs[b], axis=-1)            # [N, 128]
    grouped = probs @ GROUP_MAT                      # [N, 16] (8 contiguous cols per group)
    avg     = mean(grouped, axis=0)                  # [16]
    loss_b  = KL(softmax(targets[b]/100) || softmax(avg)) / 16
    out     = mean_b(loss_b)

Key identity: grouping+mean commute, so each core only needs
    colsum[c] = sum_n exp(x[n,c]) / rowsum[n]        # [128]
and the rest is trivial scalar math done on host.

Device kernel (per core, one batch item, data parallel over 8 cores):
  - DMA: 7 bulk groups of 4 MiB (16 KB/partition lines, ~415 GB/s) + a
    tapered tail of small groups so the compute pipeline drains with the
    data. Taper groups use dedicated SBUF buffers so their DMA triggers
    carry no buffer-recycle waits and all issue up-front.
  - ACT: exp (f32 -> bf16) in 8-row units.
  - DVE: row sums via single direct [128,8,128] reduces (634ns each —
    cheaper than any add-tree at this granularity), one reciprocal per
    group.
  - PE : per 4-row chunk, psum[0:4, 0:512] += rb^T @ exp (diagonal blocks
    hold the real reciprocal-weighted colsums). Two accumulation chains:
    bank A (bulk) closes early so its PSUM copy + output DMA overlap the
    taper; bank B (taper) ships at the end.
  - out: colsum [4, 2*4*128] f32 -> DRAM; host sums the two banks'
    diagonal blocks and finishes the tiny KL math in numpy.
"""

import numpy as np

B = 8
N = 65536
C = 128
G = 16
P = 128
K = 32          # rows per partition per group tile
NG = N // (P * K)   # 16 groups
EPS = 1e-8

_compiled = None


def _patch_tile_epilogue(tile):
    """Replace TileContext's end-of-kernel drain+barrier with a lighter one:
    the sync.drain already waits on the global completion clock, so the two
    all-engine barriers around the semaphore clears only need sequencer-level
    (sem_only) sync — the per-engine InstDrains they normally emit cost ~9us.
    The semaphore clear itself is kept: it is what guarantees a clean device
    for the next execution (removing it was measured to occasionally corrupt
    the FIRST run of a fresh process after a dirty predecessor — NaN)."""
    if getattr(tile.TileContext, "_fast_epilogue", False):
        return
    from concourse.vector_clock import ScopedClock

    def _drain_and_barrier(self, tick_clock, wait_clock):
        drain_inst = self.nc.sync.drain()
        wait_clock.add_sem_waits(
            drain_inst.ins, ScopedClock({None: tick_clock.global_clock})
        )
        self.nc.all_engine_barrier(sem_only=True)
        popped = self.nc._tile_sem_poison_stack.pop()
        assert popped is self._sem_poison
        self.nc.clear_and_free_semaphores(list(self.sems.allocated().values()))
        self.nc.all_engine_barrier(sem_only=True)

    tile.TileContext._drain_and_barrier = _drain_and_barrier
    tile.TileContext._fast_epilogue = True


def _build(ng: int = NG):
    import concourse.bacc as bacc
    import concourse.bass as bass
    import concourse.tile as tile
    from concourse import mybir

    _patch_tile_epilogue(tile)

    f32 = mybir.dt.float32
    bf16 = mybir.dt.bfloat16

    n = P * K * ng

    nc = bacc.Bacc(
        "TRN2",
        target_bir_lowering=False,
        debug=False,
        num_devices=B,
    )

    x = nc.dram_tensor("x", [n, C], f32, kind="ExternalInput")
    colsum = nc.dram_tensor("colsum", [4, 2 * 4 * C], f32, kind="ExternalOutput")

    # DMA group schedule: bulk groups of K rows/partition, then a gentle
    # taper so the compute pipeline (exp -> rowsum -> matmul) drains along
    # with the data. Compute is further split into units of <= KU rows so
    # each pipeline stage's per-unit latency is small: the serial drain
    # after the last byte lands is one small unit's chain, not a 2 MiB
    # group's worth of exp+reduce+matmul.
    # 2 MiB bulk groups: 16 KB/partition lines already run at the ~415 GB/s
    # roofline, and the smaller group keeps exp's start latency (it waits
    # for the whole group's DMA) at ~5us instead of ~10us, which shrinks
    # the compute backlog at the bulk/taper boundary.
    KB = 32
    # Graded taper: measured (A/B on HW) ~0.4us faster than a uniform
    # [8]*8 taper — the shrinking final groups (4 rows) keep the very last
    # exp->reduce->matmul chain minimal.
    if ng == NG:
        specs = [KB] * 14 + [16, 16, 8, 8, 8, 4, 4]
    else:
        specs = [K] * ng
    assert sum(specs) == ng * K

    MB = 4   # matmul chunk block: MB chunks of 128 rows per matmul instruction
    # 8-row compute units: a direct [128,8,128] reduce costs 634ns, so two
    # of them (1.27us/16 rows) beat the 16-row add-tree (~1.74us) — the
    # add-tree path below becomes dead and total DVE work drops ~25%.
    KU = 8

    # compute units: (group, offset_rows, unit_rows)
    units = []
    for g, kk in enumerate(specs):
        o = 0
        while o < kk:
            u = min(KU, kk - o)
            units.append((g, o, u))
            o += u
    # Taper-group units accumulate into PSUM bank B so bank A's chain closes
    # at the end of the bulk phase: its PSUM->SBUF copy and output DMA then
    # overlap the taper instead of queueing at the very end.
    nbulk = sum(1 for g, kk in enumerate(specs) if kk == KB)
    tail_unit = lambda g: g >= nbulk
    nA = sum(max(1, u // MB) for (g, _, u) in units if not tail_unit(g))
    nB = sum(max(1, u // MB) for (g, _, u) in units if tail_unit(g))

    with tile.TileContext(nc) as tc:
        with (
            tc.tile_pool(name="xin", bufs=6) as xpool,
            tc.tile_pool(name="xtap", bufs=1) as tpool,
            tc.tile_pool(name="exp", bufs=12) as epool,
            tc.tile_pool(name="half", bufs=4) as hpool,
            tc.tile_pool(name="small", bufs=12) as spool,
            tc.tile_pool(name="out", bufs=1) as opool,
            tc.tile_pool(name="psum", bufs=2, space="PSUM") as ppool,
        ):
            psA = ppool.tile([MB, MB * C], f32, tag="psA")
            psB = ppool.tile([MB, MB * C], f32, tag="psB")
            ot = opool.tile([MB, 2 * MB * C], f32)

            xts = {}
            row0 = 0
            for g, kk in enumerate(specs):
                src = (
                    x.ap()[row0 : row0 + P * kk, :]
                    .rearrange("(p k) c -> p (k c)", p=P, k=kk)
                )
                if kk == KB:
                    # Two 1 MiB half-DMAs per group: the overlap tracker is
                    # slice-precise, so each 16-row exp unit depends only on
                    # its own half — exp starts ~2.5us earlier than with one
                    # whole-group transfer, shrinking the ACT backlog that
                    # gates the post-stream tail.
                    xt = xpool.tile([P, kk * C], f32, tag="x")
                    h = (kk // 2) * C
                    nc.sync.dma_start(out=xt[:, 0:h], in_=src[:, 0:h])
                    nc.sync.dma_start(out=xt[:, h : kk * C], in_=src[:, h : kk * C])
                else:
                    # Taper groups get dedicated buffers so their DMA
                    # triggers carry no buffer-recycle waits: all issue
                    # up-front and the SDMA queue streams bulk -> taper
                    # back-to-back with no issue-side gaps.
                    xt = tpool.tile([P, kk * C], f32, tag=f"t{g}")
                    nc.sync.dma_start(out=xt[:], in_=src)
                xts[g] = xt
                row0 += P * kk

            mmiA = mmiB = 0
            gr_st = gr_rb = None
            gr_first = False
            for ui, (g, o, u) in enumerate(units):
                xt = xts[g]
                kk = specs[g]
                tail = tail_unit(g)
                et = epool.tile([P, u * C], bf16, tag="e")
                nc.scalar.activation(
                    et[:],
                    xt[:, o * C : (o + u) * C],
                    mybir.ActivationFunctionType.Exp,
                )

                e3 = et[:].rearrange("p (k c) -> p k c", c=C)
                # One st/rb tile per GROUP: each unit's reduce writes its
                # slice, then a single reciprocal covers the whole group
                # (reciprocal has ~200ns fixed cost; batching halves it).
                if o == 0:
                    gr_st = spool.tile([P, kk], f32, tag="s")
                    gr_rb = spool.tile([P, kk], bf16, tag="rb")
                st = gr_st[:, o : o + u]
                if u <= 8:
                    # small unit: single reduce beats 3 instructions
                    nc.vector.reduce_sum(st, e3, axis=mybir.AxisListType.X)
                else:
                    at = hpool.tile([P, u * 64], bf16, tag="a")
                    a3 = at[:].rearrange("p (k c) -> p k c", c=64)
                    nc.vector.tensor_add(a3, e3[:, :, 0:64], e3[:, :, 64:128])
                    bt = hpool.tile([P, u * 32], bf16, tag="b")
                    b3 = bt[:].rearrange("p (k c) -> p k c", c=32)
                    nc.vector.tensor_add(b3, a3[:, :, 0:32], a3[:, :, 32:64])
                    nc.vector.reduce_sum(st, b3, axis=mybir.AxisListType.X)
                if o + u == kk:
                    with nc.allow_low_precision("bf16 reciprocal weights"):
                        nc.vector.reciprocal(gr_rb[:], gr_st[:])

                for k0 in range(o, o + u, MB):
                    m = min(MB, o + u - k0)
                    if tail:
                        nc.tensor.matmul(
                            psB[0:m, 0 : m * C],
                            gr_rb[:, k0 : k0 + m],
                            et[:, (k0 - o) * C : (k0 - o + m) * C],
                            start=(mmiB == 0),
                            stop=(mmiB == nB - 1),
                        )
                        mmiB += 1
                    else:
                        nc.tensor.matmul(
                            psA[0:m, 0 : m * C],
                            gr_rb[:, k0 : k0 + m],
                            et[:, (k0 - o) * C : (k0 - o + m) * C],
                            start=(mmiA == 0),
                            stop=(mmiA == nA - 1),
                        )
                        mmiA += 1
                if mmiA == nA and not tail:
                    # Bank A's chain closed at the end of the bulk phase:
                    # copy+ship it now, overlapped with the taper. ACT does
                    # the PSUM read — DVE is the busy engine here.
                    nc.scalar.copy(ot[:, 0 : MB * C], psA[:])
                    nc.sync.dma_start(
                        out=colsum[:, 0 : MB * C], in_=ot[:, 0 : MB * C]
                    )
            assert mmiA == nA and mmiB == nB, (mmiA, nA, mmiB, nB)

            # Diagonal blocks of psA/psB hold the real partial colsums;
            # off-diagonal blocks are accumulation garbage. Ship both banks
            # and let the host pick the diagonals (engines can't start at
            # partition>0, and DMA can't read PSUM directly).
            nc.scalar.copy(ot[:, MB * C : 2 * MB * C], psB[:])
            nc.sync.dma_start(
                out=colsum[:, MB * C : 2 * MB * C], in_=ot[:, MB * C : 2 * MB * C]
            )

    nc.compile()
    return nc


def _get_compiled():
    global _compiled
    if _compiled is None:
        _compiled = _build()
    return _compiled


def _run_device(inputs: np.ndarray, trace: bool = False, **kwargs):
    from concourse.bass_utils import run_bass_kernel_spmd

    nc = _get_compiled()
    in_maps = [
        {"x": np.ascontiguousarray(inputs[i], dtype=np.float32)} for i in range(B)
    ]
    res = run_bass_kernel_spmd(nc, in_maps, list(range(B)), trace=trace, **kwargs)
    # colsum is [4, 2*4*C]: two PSUM banks side by side, each [4, 4*C] whose
    # 4 diagonal [1, C] blocks are real partial colsums.
    colsums = np.stack(
        [
            np.asarray(res.results[i]["colsum"], dtype=np.float64)
            .reshape(4, 2, 4, C)[np.arange(4), :, np.arange(4)]
            .sum(axis=(0, 1))
            for i in range(B)
        ]
    )  # [B, 128]
    return colsums, res


def _finish_host(colsums: np.ndarray, targets: np.ndarray) -> np.ndarray:
    # colsums: [B, 128] float; targets: [B, 16]
    cs = colsums.astype(np.float64)
    avg = cs.reshape(B, G, C // G).sum(axis=-1) / N          # [B, 16]
    # softmax(avg)
    a = avg - avg.max(axis=-1, keepdims=True)
    p = np.exp(a)
    p /= p.sum(axis=-1, keepdims=True)
    # softmax(targets / 100)
    t = targets.astype(np.float64) / 100.0
    t = t - t.max(axis=-1, keepdims=True)
    t = np.exp(t)
    t /= t.sum(axis=-1, keepdims=True)
    log_p = np.log(p + EPS)
    kl = (t * (np.log(t) - log_p)).sum(axis=-1) / G          # [B]
    return np.float32(kl.mean())


def kernel(inputs: np.ndarray, targets: np.ndarray) -> np.ndarray:
    inputs = np.asarray(inputs)
    colsums = None
    for _attempt in range(3):
        colsums, _ = _run_device(inputs)
        # Invariant: sum_c colsum[c] = sum_n sum_c exp(x)/rowsum = N exactly
        # (up to bf16 rounding, ~0.5%). The first execution after a dirty
        # device state (killed predecessor / congested upload) occasionally
        # returns garbage or NaN — detect and retry.
        tot = colsums.sum(axis=1)
        if np.all(np.isfinite(tot)) and np.all(np.abs(tot - N) < 0.05 * N):
            break
    return _finish_host(colsums, np.asarray(targets))

